# revision 1
# baseline (speedup 1.0000x reference)
"""Luong-style attention (B=16, T=S=E=D=1024) on 8 TRN2 NeuronCores.

Data-parallel over batch: 2 batches per core, no collectives. Per batch:

    M1   = H @ A            (T,E)     [A = W_attn]  bf16 operands
    G    = M1 @ Enc^T       (T,S)     fp32r; energies minus the H@b term
    ener = G + (H@b)[:,None]          (output attn_energies)
    W    = softmax_rows(G)            (== softmax(ener); bias is row-constant)
    C^T  = Enc(stationary) @ W^T      weighted context, transposed on chip
    h    = tanh([C|H] @ W_out^T)      via lhsT = [C^T; H^T]

mm1 runs on bf16-rounded operands (A, H^T) while mm2 keeps fp32r
(M1 carried in an f32r container, Enc^T DMA'd fp32): this halves the
DMA-critical startup bytes (no 4MB fp32 A, no 8MB fp32 H^T) at a
measured cost of attn_weights rel err 1.11e-2 (vs 1.05e-3 all-f32r),
deterministic and 1.8x under the 2e-2 gate.

Over the 379-394us baseline (same math), this version software-
pipelines the PE instruction stream so the PE never waits on the
softmax chain: the transposes of t-tile k are issued after the score
matmuls of t-tile k+1, the NEXT block's mm1 fills the end-of-block
softmax tail (the last block instead pre-computes its mm4 H-part into
the free psG banks, with the C-part joining the same PSUM groups after
mm3), the row max is reduced per 512-half so it starts before the
second score group finishes, exp emits bf16 directly, the identity /
warmup tiles come from the host instead of gpsimd memset, H^T lives in
a rotating bf16 double-buffer (current block's mm4 / next block's mm1),
and the first block's mm1 is restructured dt-outer so compute starts as
soon as the first ~400KB of DMA lands rather than after 3MB. All DMAs
are issued in dependency-resolution order because the SP HWDGE ring is
FIFO and a waiting DMA blocks everything behind it.

Two dead ends, for the record: (1) the PE transpose ignores the VALUES
of the moving operand - folding 1/Z into it via a diagonal matrix
silently produces unnormalized weights; (2) SBUF->SBUF XBAR DMA
transpose (dma_start(transpose=True)) crashes the device with
NRT_EXEC_UNIT_UNRECOVERABLE.

Measured on the 8-core chip: 317-320us HW exec (occasionally ~378us
when the chip's power manager holds the PE at 2.0GHz instead of 2.4),
rel err 1.11e-2 (deterministic), PE busy >90% at the N=512 matmul
issue-rate roofline (~210ns/MM measured), with ~20us of fixed runtime
startup/teardown inside the measured window.
"""

import os
import numpy as np
import ml_dtypes

B, T, S, E, D = 16, 1024, 1024, 1024, 1024
P = 128
NCORES = 8
BPC = B // NCORES
TH = 2
THS = T // TH
ET = E // P
DT = D // P
ST = S // P
TT = T // P
CT = (E + D) // P
NBLK = BPC * TH
TLN = THS // P  # t-tiles per block

BF16 = ml_dtypes.bfloat16

TRACE = bool(os.environ.get("BASS_KERNEL_TRACE"))
LAST_EXEC_NS = None
_cached = None


def _install_trace_shim():
    import sys, types
    import antenv
    if getattr(antenv, "axon_hooks", None) is not None:
        return
    mod = types.ModuleType("antenv.axon_hooks")
    state = {"hook": None}
    mod.set_axon_ntff_profile_hook = lambda h: state.__setitem__("hook", h)
    mod.get_axon_ntff_profile_hook = lambda: state["hook"]
    sys.modules["antenv.axon_hooks"] = mod
    antenv.axon_hooks = mod
    try:
        from trn_agent_boot.trn_boot import _ntff_profile_via_ctypes
        mod.set_axon_ntff_profile_hook(
            _ntff_profile_via_ctypes("/opt/axon/libaxon_pjrt.so"))
    except Exception:
        pass
    import concourse.bass_utils as bu
    bu.upload_artifacts = lambda tmpdir: "local://" + tmpdir


def _build():
    import concourse.bass as bass
    import concourse.bacc as bacc
    import concourse.mybir as mybir
    import concourse.tile as tile
    from contextlib import ExitStack

    dt = mybir.dt
    ts = bass.ts
    AF = mybir.ActivationFunctionType

    nc = bacc.Bacc("TRN2", target_bir_lowering=False, debug=False)

    ident_d = nc.declare_dram_parameter("ident_d", [P, P], dt.bfloat16, isOutput=False)
    A_bf = nc.declare_dram_parameter("A_bf", [D, E], dt.bfloat16, isOutput=False)
    WoT = nc.declare_dram_parameter("WoT", [E + D, D], dt.bfloat16, isOutput=False)
    HT_bfd = nc.declare_dram_parameter("HT_bfd", [BPC, D, T], dt.bfloat16, isOutput=False)
    EncT_r = nc.declare_dram_parameter("EncT_r", [BPC, E, S], dt.float32r, isOutput=False)
    Enc = nc.declare_dram_parameter("Enc", [BPC, S, E], dt.bfloat16, isOutput=False)
    hb = nc.declare_dram_parameter("hb", [BPC, T], dt.float32, isOutput=False)
    out_h = nc.declare_dram_parameter("out_h", [BPC, T, D], dt.float32, isOutput=True)
    out_w = nc.declare_dram_parameter("out_w", [BPC, T, S], dt.float32, isOutput=True)
    out_e = nc.declare_dram_parameter("out_e", [BPC, T, S], dt.float32, isOutput=True)

    with tile.TileContext(nc) as tc, ExitStack() as ctx:
        const = ctx.enter_context(tc.tile_pool(name="const", bufs=1))
        wpool = ctx.enter_context(tc.tile_pool(name="wpool", bufs=1))
        bpool = ctx.enter_context(tc.tile_pool(name="bpool", bufs=1))
        hpool = ctx.enter_context(tc.tile_pool(name="hpool", bufs=1))
        work = ctx.enter_context(tc.tile_pool(name="work", bufs=2))
        psA = ctx.enter_context(tc.tile_pool(name="psA", bufs=2, space="PSUM"))
        psG = ctx.enter_context(tc.tile_pool(name="psG", bufs=2, space="PSUM"))
        psT = ctx.enter_context(tc.tile_pool(name="psT", bufs=2, space="PSUM"))

        # ---- persistent SBUF tensors
        ident = const.tile([P, P], dt.bfloat16)
        warm_a = const.tile([P, P], dt.bfloat16)
        warm = const.tile([P, 512], dt.bfloat16)
        # memset (gpsimd) instead of DMA: the PE can start its warmup
        # ~2us before the DMA ring delivers its first bytes
        nc.gpsimd.memset(warm_a[:], 0.0)
        nc.gpsimd.memset(warm[:], 0.0)
        a_bf = wpool.tile([P, DT, E], dt.bfloat16)
        wo = wpool.tile([P, CT, D], dt.bfloat16)
        hb_sb = wpool.tile([P, BPC, TT], dt.float32)
        enc_sb = bpool.tile([P, ST, E], dt.bfloat16, tag="enc")
        encT_r = bpool.tile([P, ET, S], dt.float32r, tag="encT")
        m1_r = hpool.tile([P, ET, THS], dt.float32r, tag="m1_r")
        wt_sb = hpool.tile([P, ST, THS], dt.bfloat16, tag="wt")
        ct_sb = hpool.tile([P, ET, THS], dt.bfloat16, tag="ct")

        def ht_tile(name):
            # two live H^T slices: the current block's (mm4) and the next
            # block's (its mm1, inlined into the current block)
            return hpool.tile([P, DT, THS], dt.bfloat16, tag="ht_bf",
                              bufs=2, name=name)

        a_ap = A_bf.ap().rearrange("(dtt p) e -> p dtt e", p=P)
        encT_ap = lambda b: EncT_r.ap()[b].rearrange("(et p) s -> p et s", p=P)
        enc_ap = lambda b: Enc.ap()[b].rearrange("(st p) e -> p st e", p=P)
        ht_ap = lambda b: HT_bfd.ap()[b].rearrange("(dtt p) t -> p dtt t", p=P)

        # ---- startup DMAs: the SP ring is FIFO, so order == consumption
        # order, criticals first.
        ht_cur = ht_tile("ht0")
        ht_next = ht_tile("ht1")
        with tc.high_priority():
            nc.sync.dma_start(ident[:], ident_d.ap())
            # mm1 of block 0 consumes (a_bf chunk dti) x (ht chunk dti)
            for dti in range(DT):
                nc.sync.dma_start(a_bf[:, dti, :], a_ap[:, dti, :])
                nc.sync.dma_start(ht_cur[:, dti, :],
                                  ht_ap(0)[:, dti, ts(0, THS)])
            # G of block 0 consumes encT chunks sc-outer, then et: the
            # first score group only needs the sc=0 halves, so it starts
            # 2MB (~7us) before the full tensor lands
            for sc in range(2):
                for et in range(ET):
                    nc.sync.dma_start(encT_r[:, et, ts(sc, 512)],
                                      encT_ap(0)[:, et, ts(sc, 512)])
            for b in range(BPC):
                nc.sync.dma_start(hb_sb[:, b, :],
                                  hb.ap()[b].rearrange("(tt p) -> p tt", p=P))

        def warm_fill(n, name):
            """Dummy matmuls that keep the PE busy (and HAM un-throttled)
            across a known DMA-paced or cross-engine-latency window."""
            wps = psA.tile([P, 512], dt.float32, tag="psA", name=name)
            for wi in range(n):
                nc.tensor.matmul(wps[:], warm_a[:], warm[:],
                                 start=(wi == 0), stop=(wi == n - 1))

        # (no dedicated warmup: the first mm1 chunks land ~3us before the
        # PE engine finishes booting, so the real dt-outer mm1 rounds are
        # compute-bound through the HAM cold window themselves)

        # ---- block 0 mm1, dt-outer so it's paced by the chunked DMAs:
        # four [P,512] accumulators live in the psG pool (2 tiles x 2 banks),
        # each holding two adjacent et outputs.
        for half in range(2):
            accs = [psG.tile([P, 1024], dt.float32, tag="psG",
                             name=f"mm1acc_{half}_{i}") for i in range(2)]
            for dti in range(DT):
                for ei in range(4):
                    et = half * 4 + ei
                    acc = accs[ei // 2]
                    nc.tensor.matmul(acc[:, ts(ei % 2, 512)],
                                     a_bf[:, dti, ts(et, P)],
                                     ht_cur[:, dti, :],
                                     start=(dti == 0), stop=(dti == DT - 1))
            for ei in range(4):
                et = half * 4 + ei
                nc.vector.tensor_copy(m1_r[:, et, :],
                                      accs[ei // 2][:, ts(ei % 2, 512)])

        # remaining inputs, in deadline order (each is dep-free or its WAR
        # dep resolves before anything queued behind it is needed)
        nc.sync.dma_start(ht_next[:, :, :], ht_ap(0)[:, :, ts(1, THS)])
        nc.sync.dma_start(enc_sb[:], enc_ap(0))
        nc.sync.dma_start(wo[:], WoT.ap().rearrange("(ct p) d -> p ct d", p=P))

        def tr_phase(st8):
            """Transpose one softmax'd t-tile into wt_sb on the PE.
            (An XBAR DMA-transpose version of this crashed the device with
            NRT_EXEC_UNIT_UNRECOVERABLE — SBUF->SBUF DMA transpose appears
            unusable here, so the transposes stay on the PE.)"""
            wbf_t, tl = st8
            for g in range(2):
                trp = psT.tile([P, 512], dt.bfloat16, tag="psT")
                for k in range(4):
                    st = g * 4 + k
                    nc.tensor.transpose(trp[:, ts(k, P)],
                                        wbf_t[:, ts(st, P)], ident[:])
                nc.vector.tensor_copy(
                    wt_sb[:, g * 4:(g + 1) * 4, ts(tl, P)], trp[:])

        def softmax_issue(b, th, tl, G):
            tt = th * TLN + tl
            # split max: the first half only depends on the sc=0 matmul
            # group, so it runs ~1.7us before G is fully done
            nmax_a = work.tile([P, 1], dt.float32, tag="nmax_a", bufs=3)
            nc.vector.reduce_max(nmax_a[:], G[:, ts(0, 512)],
                                 axis=mybir.AxisListType.X, negate=True)
            negmax = work.tile([P, 1], dt.float32, tag="negmax", bufs=3)
            nc.vector.reduce_max(negmax[:], G[:, ts(1, 512)],
                                 axis=mybir.AxisListType.X, negate=True)
            nc.vector.tensor_scalar_min(negmax[:], in0=negmax[:],
                                        scalar1=nmax_a[:])
            pexp = work.tile([P, S], dt.bfloat16, tag="pexp", bufs=3)
            sume = work.tile([P, 1], dt.float32, tag="sume", bufs=3)
            nc.scalar.activation(pexp[:], G[:], AF.Exp,
                                 bias=negmax[:], scale=1.0,
                                 accum_out=sume[:])
            rec = work.tile([P, 1], dt.float32, tag="rec", bufs=3)
            nc.vector.reciprocal(rec[:], sume[:])
            wbf = work.tile([P, S], dt.bfloat16, tag="wbf", bufs=3)
            nc.vector.tensor_scalar_mul(wbf[:], in0=pexp[:], scalar1=rec[:])
            ener = work.tile([P, S], dt.float32, tag="ener")
            nc.scalar.activation(ener[:], G[:], AF.Identity,
                                 bias=hb_sb[:, b, tt:tt + 1], scale=1.0)
            nc.sync.dma_start(out_e.ap()[b, ts(tt, P), :], ener[:])
            wexp = work.tile([P, S], dt.float32, tag="wexp")
            nc.vector.tensor_scalar_mul(wexp[:], in0=pexp[:], scalar1=rec[:])
            nc.sync.dma_start(out_w.ap()[b, ts(tt, P), :], wexp[:])
            return wbf, tl

        # ---- main loop over blocks; block blk's mm1 already ran (block 0:
        # above; others: inlined into the previous block). ht_cur holds this
        # block's H^T (for mm4), ht_next the next block's (for its mm1).
        for blk in range(NBLK):
            b, th = blk // TH, blk % TH
            nb, nth = (blk + 1) // TH, (blk + 1) % TH

            # ---- score matmuls + softmax, transposes pipelined one tile back
            pend = None
            for tl in range(TLN):
                G = psG.tile([P, S], dt.float32, tag="psG")
                for sc in range(2):
                    for et in range(ET):
                        nc.tensor.matmul(
                            G[:, ts(sc, 512)],
                            m1_r[:, et, ts(tl, P)],
                            encT_r[:, et, ts(sc, 512)],
                            start=(et == 0), stop=(et == ET - 1))
                    if blk == 0 and tl < 3:
                        # block 0's G is paced by the encT chunk DMAs until
                        # roughly the end of its third t-tile (~23us at the
                        # observed ~300GB/s); small dummy fills absorb the
                        # pacing gaps so HAM never re-throttles
                        warm_fill(2, f"warmg_{tl}_{sc}")
                cur = softmax_issue(b, th, tl, G)
                if pend is not None:
                    tr_phase(pend)
                pend = cur

            # next batch's encoder tensors; the encT chunks' WAR deps (this
            # block's G) resolve chunk by chunk as G(3) completes.
            if th == TH - 1 and b + 1 < BPC:
                for et in range(ET):
                    nc.sync.dma_start(encT_r[:, et, :], encT_ap(b + 1)[:, et, :])

            # ---- next block's mm1 fills the softmax tail; the last
            # transpose slots between its two halves. The last block has no
            # mm1 left, so dummy matmuls bridge the softmax latency instead.
            if blk + 1 < NBLK:
                for et in range(ET // 2):
                    acc = psA.tile([P, 512], dt.float32, tag="psA")
                    for dti in range(DT):
                        nc.tensor.matmul(acc[:], a_bf[:, dti, ts(et, P)],
                                         ht_next[:, dti, :],
                                         start=(dti == 0), stop=(dti == DT - 1))
                    nc.vector.tensor_copy(m1_r[:, et, :], acc[:])
                tr_phase(pend)
                for et in range(ET // 2, ET):
                    acc = psA.tile([P, 512], dt.float32, tag="psA")
                    for dti in range(DT):
                        nc.tensor.matmul(acc[:], a_bf[:, dti, ts(et, P)],
                                         ht_next[:, dti, :],
                                         start=(dti == 0), stop=(dti == DT - 1))
                    nc.vector.tensor_copy(m1_r[:, et, :], acc[:])
            else:
                # last block: no mm1 left — bridge the last t-tile's softmax
                # + transpose latency with REAL work: the mm4 H-part for
                # t-tiles 0,1, accumulated early into the now-free psG banks
                # (the C-part joins the same PSUM groups after mm3).
                # the transpose slots between the two H-part groups: the
                # first group covers the softmax latency it waits on, and
                # it covers the ener-release the second group waits on
                hacc01 = []
                for tl2 in range(2):
                    hacc = psG.tile([P, 1024], dt.float32, tag="psG",
                                    name=f"hacc01_{tl2}")
                    for dc in range(2):
                        for ci in range(DT):
                            nc.tensor.matmul(hacc[:, ts(dc, 512)],
                                             ht_cur[:, ci, ts(tl2, P)],
                                             wo[:, ET + ci, ts(dc, 512)],
                                             start=(ci == 0), stop=False)
                    hacc01.append(hacc)
                    if tl2 == 0:
                        tr_phase(pend)

            # ---- mm3: CT[e',t] = sum_s Enc[s,e'] WT[s,t]
            for e2 in range(ET):
                cacc = psA.tile([P, 512], dt.float32, tag="psA")
                for st in range(ST):
                    nc.tensor.matmul(cacc[:], enc_sb[:, st, ts(e2, P)],
                                     wt_sb[:, st, :],
                                     start=(st == 0), stop=(st == ST - 1))
                nc.scalar.copy(ct_sb[:, e2, :], cacc[:])

            # enc_sb for the next batch: its WAR dep (this block's mm3) has
            # just been emitted, so it won't head-of-line-block the ring
            # beyond what's necessary.
            if th == TH - 1 and b + 1 < BPC:
                nc.sync.dma_start(enc_sb[:], enc_ap(b + 1))

            # ---- mm4: h[t,d] = tanh(sum_c [CT;HT][c,t] WoT[c,d]);
            # H-part first so the tail of mm3's ct copies stays off the
            # critical path.
            for tl in range(TLN):
                tt = th * TLN + tl
                if blk + 1 == NBLK and tl < 2:
                    hacc2 = hacc01[tl]
                    for dc in range(2):
                        for ci in range(ET):
                            nc.tensor.matmul(hacc2[:, ts(dc, 512)],
                                             ct_sb[:, ci, ts(tl, P)],
                                             wo[:, ci, ts(dc, 512)],
                                             start=False, stop=(ci == ET - 1))
                        h_sb = work.tile([P, 512], dt.float32, tag="h_sb")
                        nc.scalar.activation(h_sb[:], hacc2[:, ts(dc, 512)],
                                             AF.Tanh)
                        nc.sync.dma_start(
                            out_h.ap()[b, ts(tt, P), ts(dc, 512)], h_sb[:])
                    continue
                for dc in range(2):
                    hacc = psA.tile([P, 512], dt.float32, tag="psA")
                    for ci in range(DT):
                        nc.tensor.matmul(hacc[:], ht_cur[:, ci, ts(tl, P)],
                                         wo[:, ET + ci, ts(dc, 512)],
                                         start=(ci == 0), stop=False)
                    for ci in range(ET):
                        nc.tensor.matmul(hacc[:], ct_sb[:, ci, ts(tl, P)],
                                         wo[:, ci, ts(dc, 512)],
                                         start=False, stop=(ci == ET - 1))
                    h_sb = work.tile([P, 512], dt.float32, tag="h_sb")
                    nc.scalar.activation(h_sb[:], hacc[:], AF.Tanh)
                    nc.sync.dma_start(
                        out_h.ap()[b, ts(tt, P), ts(dc, 512)], h_sb[:])

            if blk + 1 < NBLK:
                # rotate the H^T double-buffer; the refill DMA sits after
                # mm4 in the SP FIFO so its WAR dep (this block's mm4 reads
                # of the buffer being recycled) is already resolved.
                ht_cur = ht_next
                if blk + 2 <= NBLK - 1:
                    nnb, nnth = (blk + 2) // TH, (blk + 2) % TH
                    ht_next = ht_tile(f"ht{blk + 2}")
                    nc.sync.dma_start(ht_next[:, :, :],
                                      ht_ap(nnb)[:, :, ts(nnth, THS)])

    nc.compile()
    return nc


def kernel(hidden, encoder_outputs, W_attn, b_attn, W_out):
    global _cached, LAST_EXEC_NS
    hidden = np.asarray(hidden, dtype=np.float32)
    encoder_outputs = np.asarray(encoder_outputs, dtype=np.float32)
    W_attn = np.asarray(W_attn, dtype=np.float32)
    b_attn = np.asarray(b_attn, dtype=np.float32)
    W_out = np.asarray(W_out, dtype=np.float32)

    if TRACE:
        _install_trace_shim()
    if _cached is None:
        _cached = _build()
    nc = _cached
    from concourse.bass_utils import run_bass_kernel_spmd

    WoT = np.ascontiguousarray(W_out.T).astype(BF16)
    hb_full = (hidden.reshape(B * T, D) @ b_attn).reshape(B, T).astype(np.float32)
    ident_np = np.eye(P, dtype=np.float32).astype(BF16)

    A_bf = W_attn.astype(BF16)
    in_maps = []
    for c in range(NCORES):
        sl = slice(BPC * c, BPC * (c + 1))
        h = hidden[sl]
        enc = encoder_outputs[sl]
        HT = np.ascontiguousarray(h.transpose(0, 2, 1))
        EncT = np.ascontiguousarray(enc.transpose(0, 2, 1))
        in_maps.append({
            "ident_d": ident_np,
            "A_bf": A_bf, "WoT": WoT,
            "HT_bfd": HT.astype(BF16),
            "EncT_r": EncT,
            "Enc": enc.astype(BF16),
            "hb": np.ascontiguousarray(hb_full[sl]),
        })

    res = run_bass_kernel_spmd(nc, in_maps, core_ids=list(range(NCORES)),
                               trace=TRACE)
    LAST_EXEC_NS = res.exec_time_ns

    h_tilde = np.concatenate([r["out_h"] for r in res.results], axis=0)
    attn_weights = np.concatenate([r["out_w"] for r in res.results], axis=0)
    attn_energies = np.concatenate([r["out_e"] for r in res.results], axis=0)
    return h_tilde, attn_weights, attn_energies

